# revision 1
# baseline (speedup 1.0000x reference)
"""ContactMapHead bilinear pair-scoring kernel for 8 trn2 NeuronCores.

Math: for each batch b, logits[b, p] = h[b, i_p] @ W @ h[b, j_p] + bias,
where (i_p, j_p) enumerate position pairs (upper triangle, k=1, when the
masks keep every position — the general case is handled too).

This equals S_b = (h_b @ W) @ h_b^T + bias followed by a pair gather.
S_b is a 512x512 matrix per batch; total device work = two 512^3 matmuls
per batch (memory-bound at this size).

Sharding (8 cores): core c computes rows [r0, r0+128) of S_b for batch
b = c // 4, r0 = (c % 4) * 128.  W and hs_b^T are replicated per core;
each core also gets its own pre-transposed row-slice hs_rows^T.  The
host assembles S (2, 512, 512) from the 8 row-blocks and gathers the
pair indices (pure unshard/reindex).

Device program (per core), P = 128 partitions, all fp32, raw bass
(manual semaphores, no Tile entry/exit barriers).  All DRAM inputs are
host-swizzled to partition-major (128, X) so every DMA descriptor is a
large contiguous run:
    w    (128, 2048): w[p, kc*512 + :] = W[kc*128 + p, :]
    hst  (128, 2048): hst[p, hc*512 + j] = hs[b, j, hc*128 + p]
    hsrt (128, 512):  hsrt[p, kc*128 + m] = hs[b, r0 + m, kc*128 + p]
    bias (1,)
    out  (128, 512):  S rows r0..r0+127 (+bias)

  stage 1 (PE): GT[hc] (128h x 128m) += lhsT=W[kc, hc-cols] x rhs=hsrt[kc]
  copy  (DVE): gt_sb[:, hc] <- GT[hc]
  stage 2 (PE): ps (128m x 512j) += lhsT=gt_sb[:, hc] x rhs=hst[hc]
  epilogue (DVE+DMA): out = ps + bias, in two column halves on two rings
"""

import numpy as np

_B, _L, _H = 2, 512, 512
_P = 128
_KC = _H // _P          # 4 contraction chunks
_GROUPS = 4             # row-blocks per batch
_RB = _L // _GROUPS     # 128 rows per core
_NCORES = 8

# Dev/profiling knobs (used by test.py only; harness leaves them alone).
TRACE = False
TRACE_KWARGS = {}
LAST_RESULTS = None

_STATE = {}


def _build_nc():
    """Build (once) the raw-bass module for one core's row-block."""
    if "nc" in _STATE:
        return _STATE["nc"]

    from concourse import bacc, mybir

    f32 = mybir.dt.float32
    nc = bacc.Bacc("TRN2", target_bir_lowering=False, debug=False)

    w_d = nc.dram_tensor("w", [_P, 2048], f32, kind="ExternalInput")
    hst_d = nc.dram_tensor("hst", [_P, 2048], f32, kind="ExternalInput")
    hsrt_d = nc.dram_tensor("hsrt", [_P, 512], f32, kind="ExternalInput")
    bias_d = nc.dram_tensor("bias", [1], f32, kind="ExternalInput")
    out_d = nc.dram_tensor("out", [_RB, _L], f32, kind="ExternalOutput")

    w_sb = nc.alloc_sbuf_tensor("w_sb", [_P, 2048], f32)
    hst_sb = nc.alloc_sbuf_tensor("hst_sb", [_P, 2048], f32)
    hsrt_sb = nc.alloc_sbuf_tensor("hsrt_sb", [_P, 512], f32)
    bias_sb = nc.alloc_sbuf_tensor("bias_sb", [_P, 1], f32)
    gt_sb = nc.alloc_sbuf_tensor("gt_sb", [_P, 512], f32)
    out_sb = nc.alloc_sbuf_tensor("out_sb", [_P, _L], f32)
    warm_sb = nc.alloc_sbuf_tensor("warm_sb", [_P, 512], f32)
    pgt = [nc.alloc_psum_tensor(f"pgt{h}", [_P, _P], f32) for h in range(_KC)]
    ps = nc.alloc_psum_tensor("ps", [_P, _L], f32)
    pwarm = nc.alloc_psum_tensor("pwarm", [_P, _L], f32)

    s_w = [nc.alloc_semaphore(f"s_w{k}") for k in range(_KC)]  # +16 each
    s_hr = nc.alloc_semaphore("s_hr")      # +16 hsrt
    s_hst = [nc.alloc_semaphore(f"s_hst{h}") for h in range(_KC)]  # +16 each
    s_bias = nc.alloc_semaphore("s_bias")  # +16 bias
    s_gt_pe = nc.alloc_semaphore("s_gt_pe")  # +1 per stage-1 group
    s_gt_v = nc.alloc_semaphore("s_gt_v")    # +1 per gt copy
    s_s = nc.alloc_semaphore("s_s")        # +1 stage-2 done
    s_out = nc.alloc_semaphore("s_out")    # +1 per epilogue half
    s_od = nc.alloc_semaphore("s_od")      # +16 per out-DMA half
    s_warm = nc.alloc_semaphore("s_warm")  # +1 warmup scratch zeroed

    _Q = _L // 4

    with nc.Block(no_gpsimd_drain=True) as block:

        @block.sync
        def _(sync):
            # critical inputs for stage 1 first
            sync.dma_start(out=hsrt_sb[:], in_=hsrt_d[:]).then_inc(s_hr, 16)
            sync.dma_start(out=w_sb[:, 0:512], in_=w_d[:, 0:512]).then_inc(
                s_w[0], 16
            )
            sync.dma_start(out=w_sb[:, 512:1024], in_=w_d[:, 512:1024]).then_inc(
                s_w[1], 16
            )
            sync.dma_start(
                out=bias_sb[:], in_=bias_d[:].to_broadcast((_P, 1))
            ).then_inc(s_bias, 16)
            for q in (0, 1):
                sync.wait_ge(s_out, q * 2 + 1)
                sync.dma_start(
                    out=out_d[:, q * 256 : q * 256 + _Q],
                    in_=out_sb[:, q * 256 : q * 256 + _Q],
                ).then_inc(s_od, 16)
            sync.wait_ge(s_od, 64)

        @block.scalar
        def _(scalar):
            scalar.dma_start(out=w_sb[:, 1024:1536], in_=w_d[:, 1024:1536]).then_inc(
                s_w[2], 16
            )
            scalar.dma_start(out=w_sb[:, 1536:2048], in_=w_d[:, 1536:2048]).then_inc(
                s_w[3], 16
            )
            for hc in range(_KC):
                scalar.dma_start(
                    out=hst_sb[:, hc * 512 : (hc + 1) * 512],
                    in_=hst_d[:, hc * 512 : (hc + 1) * 512],
                ).then_inc(s_hst[hc], 16)
            for q in (0, 1):
                scalar.wait_ge(s_out, q * 2 + 2)
                scalar.dma_start(
                    out=out_d[:, q * 256 + _Q : (q + 1) * 256],
                    in_=out_sb[:, q * 256 + _Q : (q + 1) * 256],
                ).then_inc(s_od, 16)
            scalar.wait_ge(s_od, 64)

        @block.tensor
        def _(tensor):
            # HAM warmup: keep the PE array busy on zeros so the clock gate
            # opens (1.2 -> 2.4 GHz) before the real matmuls arrive.
            # Span the input-DMA window (~5us): inputs all land together at
            # ~13us because SDMA round-robins the rings, so the PE must stay
            # busy until then or HAM re-throttles to 1.2 GHz.
            tensor.wait_ge(s_warm, 1)
            for _ in range(4):
                nc.tensor.matmul(
                    pwarm[:],
                    lhsT=warm_sb[:, 0:_P],
                    rhs=warm_sb[:],
                    start=True,
                    stop=True,
                )
            # kc-outer so round kc only needs W chunk kc (chases the DMAs)
            tensor.wait_ge(s_hr, 16)
            for kc in range(_KC):
                tensor.wait_ge(s_w[kc], 16)
                for hc in range(_KC):
                    mm = nc.tensor.matmul(
                        pgt[hc][:],
                        lhsT=w_sb[:, kc * 512 + hc * _P : kc * 512 + (hc + 1) * _P],
                        rhs=hsrt_sb[:, kc * _P : (kc + 1) * _P],
                        start=(kc == 0),
                        stop=(kc == _KC - 1),
                    )
                    if kc == _KC - 1:
                        mm.then_inc(s_gt_pe, 1)
            for hc in range(_KC):
                tensor.wait_ge(s_gt_v, hc + 1)
                tensor.wait_ge(s_hst[hc], 16)
                mm = nc.tensor.matmul(
                    ps[:],
                    lhsT=gt_sb[:, hc * _P : (hc + 1) * _P],
                    rhs=hst_sb[:, hc * 512 : (hc + 1) * 512],
                    start=(hc == 0),
                    stop=(hc == _KC - 1),
                )
            mm.then_inc(s_s, 1)

        @block.vector
        def _(vector):
            nc.vector.memset(warm_sb[:], 0.0).then_inc(s_warm, 1)
            for hc in range(_KC):
                vector.wait_ge(s_gt_pe, hc + 1)
                nc.vector.tensor_copy(
                    gt_sb[:, hc * _P : (hc + 1) * _P], pgt[hc][:]
                ).then_inc(s_gt_v, 1)
            vector.wait_ge(s_s, 1)
            vector.wait_ge(s_bias, 16)
            # quarters, alternating sync/scalar consumers so the two out-DMA
            # rings start as early as possible and receipts pipeline
            for q in range(4):
                nc.vector.tensor_scalar_add(
                    out_sb[:, q * _Q : (q + 1) * _Q],
                    ps[:, q * _Q : (q + 1) * _Q],
                    bias_sb[:, 0:1],
                ).then_inc(s_out, 1)

    nc.compile()
    _STATE["nc"] = nc
    return nc


def _swizzle(a):
    """(512, X) row-major -> (128, 4*X): partition p gets rows p, 128+p, ..."""
    x = a.shape[1]
    return np.ascontiguousarray(
        a.reshape(_KC, _P, x).transpose(1, 0, 2).reshape(_P, _KC * x)
    )


def _device_scores(hs, w, bias):
    """Compute S[b, i, j] = (hs_b @ W @ hs_b^T)[i, j] + bias on 8 cores."""
    global LAST_RESULTS
    from concourse.bass_utils import run_bass_kernel_spmd

    nc = _build_nc()

    w_p = _swizzle(w)
    hst = [np.ascontiguousarray(hs[b].T) for b in range(_B)]
    hst_p = [_swizzle(h) for h in hst]
    in_maps = []
    for c in range(_NCORES):
        b, rc = divmod(c, _GROUPS)
        r0 = rc * _RB
        in_maps.append(
            {
                "w": w_p,
                "hst": hst_p[b],
                "hsrt": _swizzle(hst[b][:, r0 : r0 + _RB]),
                "bias": bias,
            }
        )

    kwargs = dict(TRACE_KWARGS) if TRACE else {}
    res = run_bass_kernel_spmd(
        nc, in_maps, core_ids=list(range(_NCORES)), trace=TRACE, **kwargs
    )
    LAST_RESULTS = res

    s = np.empty((_B, _L, _L), np.float32)
    for c in range(_NCORES):
        b, rc = divmod(c, _GROUPS)
        s[b, rc * _RB : (rc + 1) * _RB, :] = res.results[c]["out"]
    return s


def kernel(hidden_states, W, b, attention_mask, special_tokens_mask):
    hs = np.ascontiguousarray(np.asarray(hidden_states, dtype=np.float32))
    w = np.ascontiguousarray(np.asarray(W, dtype=np.float32)[0])
    bias = np.asarray(b, dtype=np.float32).reshape(1)
    am = np.asarray(attention_mask)
    sm = np.asarray(special_tokens_mask)

    # Pair indices from the (constant) masks — mirrors the reference.
    aa_mask = (am[0] == 1) & (sm[0] == 0)
    aa_positions = np.nonzero(aa_mask)[0]
    n_aa = aa_positions.shape[0]
    if n_aa < 2:
        return np.zeros((hs.shape[0], 0), dtype=np.float32)
    tri_i, tri_j = np.triu_indices(n_aa, k=1)
    idx_i = aa_positions[tri_i]
    idx_j = aa_positions[tri_j]

    if hs.shape != (_B, _L, _H) or w.shape != (_H, _H):
        # Defensive fallback for unexpected shapes (never hit by the spec).
        g = hs @ w
        s = np.einsum("bik,bjk->bij", g, hs) + bias[0]
        return s[:, idx_i, idx_j].astype(np.float32)

    s = _device_scores(hs, w, bias)  # bias already added on device
    return s[:, idx_i, idx_j].astype(np.float32)



# revision 4
# speedup vs baseline: 1.2284x; 1.2284x over previous
"""ContactMapHead bilinear pair-scoring kernel for 8 trn2 NeuronCores.

Math: for each batch b, logits[b, p] = h[b, i_p] @ W @ h[b, j_p] + bias,
where (i_p, j_p) enumerate position pairs (upper triangle, k=1, when the
masks keep every position — the general case is handled too).

This equals S_b = (h_b @ W) @ h_b^T + bias followed by a pair gather.
S_b is a 512x512 matrix per batch; total device work = two 512^3 matmuls
per batch (memory-bound at this size).

Sharding (8 cores): core c computes rows [r0, r0+128) of S_b for batch
b = c // 4, r0 = (c % 4) * 128.  W and h_b^T are replicated per core; the
host assembles S (2, 512, 512) from the 8 row-blocks and gathers the
pair indices (pure unshard/reindex).

v2 (this file): all inputs in bf16 — the kernel is aggregate-HBM-DMA
bound (all 8 cores share the chip's 16 SDMA engines at ~2.5 TB/s), so
halving input bytes halves the dominant phase; bf16 also makes every
matmul single-pass (fp32 runs LOW/HIGH double-pumped on the PE).
The row-slice tensor (stage-1 rhs) is folded into hst by a per-core
column permutation (cols r0..r0+127 first within each 512-col h-chunk),
so it is not transferred twice; the host inverts the permutation when
assembling S.  DMA layout keeps 2KB-per-partition runs (SDMA throughput
is per-packet limited; runs below 2KB waste packet slots).

Device program (per core), P = 128 partitions, raw bass:
    w    (128, 2048) bf16: w[p, kc*512 + c] = W[kc*128 + p, c]
    hst  (128, 2048) bf16: hst[p, hc*512 + j] = h_b[perm[j], hc*128 + p]
                           perm = [r0..r0+128) ++ [0..r0) ++ [r0+128..512)
    bias (1,) f32
    out  (128, 512) f32:  out[m, j] = S_b[r0 + m, perm[j]] + bias

  DMAs: sync ring: w front half (kc 0,1), w back half (kc 2,3)
        scalar ring: hst front (hc 0,1), hst back (hc 2,3), bias
        (each a single 2KB-run transfer; fronts land ~halfway through)
  PE:   warmup MMs on zeros (opens the HAM clock gate 1.2 -> 2.4 GHz),
        stage 1 GT[hc] += W-tile^T x hst[kc-chunk cols 0:128] with
        kc 0,1 on the front halves (overlaps the back-half DMA), then
        kc 2,3 interleaved with stage 2 ps += gt[hc] x hst[hc-chunk].
  DVE:  gt psum -> sbuf bf16 copies, then ps + bias in two column
        halves feeding two out DMAs on the two rings.
"""

import numpy as np

_B, _L, _H = 2, 512, 512
_P = 128
_KC = _H // _P          # 4 contraction chunks
_GROUPS = 4             # row-blocks per batch
_RB = _L // _GROUPS     # 128 rows per core
_NCORES = 8
_NWARM = 8              # warmup matmuls (each ~0.2-0.4us depending on clock)

# Dev/profiling knobs (used by test.py only; harness leaves them alone).
TRACE = False
TRACE_KWARGS = {}
LAST_RESULTS = None

_STATE = {}


def _build_nc():
    """Build (once) the raw-bass module for one core's row-block."""
    if "nc" in _STATE:
        return _STATE["nc"]

    from concourse import bacc, mybir

    f32 = mybir.dt.float32
    bf16 = mybir.dt.bfloat16
    nc = bacc.Bacc("TRN2", target_bir_lowering=False, debug=False)

    w_d = nc.dram_tensor("w", [_P, 2048], bf16, kind="ExternalInput")
    hst_d = nc.dram_tensor("hst", [_P, 2048], bf16, kind="ExternalInput")
    bias_d = nc.dram_tensor("bias", [1], f32, kind="ExternalInput")
    out_d = nc.dram_tensor("out", [_RB, _L], f32, kind="ExternalOutput")

    w_sb = nc.alloc_sbuf_tensor("w_sb", [_P, 2048], bf16)
    hst_sb = nc.alloc_sbuf_tensor("hst_sb", [_P, 2048], bf16)
    bias_sb = nc.alloc_sbuf_tensor("bias_sb", [_P, 1], f32)
    gt_sb = nc.alloc_sbuf_tensor("gt_sb", [_P, 512], bf16)
    out_sb = nc.alloc_sbuf_tensor("out_sb", [_P, _L], f32)
    warm_sb = nc.alloc_sbuf_tensor("warm_sb", [_P, _P], bf16)
    pgt = [nc.alloc_psum_tensor(f"pgt{h}", [_P, _P], f32) for h in range(_KC)]
    ps = nc.alloc_psum_tensor("ps", [_P, _L], f32)
    pwarm = nc.alloc_psum_tensor("pwarm", [_P, _P], f32)

    s_wf = nc.alloc_semaphore("s_wf")      # +16 w front
    s_wb = nc.alloc_semaphore("s_wb")      # +16 w back
    s_hf = nc.alloc_semaphore("s_hf")      # +16 hst front
    s_hb = nc.alloc_semaphore("s_hb")      # +16 hst back
    s_bias = nc.alloc_semaphore("s_bias")  # +16 bias
    s_gt_pe = nc.alloc_semaphore("s_gt_pe")  # +1 per stage-1 group done
    s_gt_v = nc.alloc_semaphore("s_gt_v")    # +1 per gt copy
    s_s2 = nc.alloc_semaphore("s_s2")      # +1 stage-2 done
    s_out = nc.alloc_semaphore("s_out")    # +1 per bias-add half
    s_od = nc.alloc_semaphore("s_od")      # +16 per out-DMA half
    s_warm = nc.alloc_semaphore("s_warm")  # +1 warmup scratch zeroed

    with nc.Block(no_gpsimd_drain=True) as block:

        @block.sync
        def _(sync):
            sync.dma_start(out=w_sb[:, 0:1024], in_=w_d[:, 0:1024]).then_inc(
                s_wf, 16
            )
            sync.dma_start(out=w_sb[:, 1024:2048], in_=w_d[:, 1024:2048]).then_inc(
                s_wb, 16
            )
            # out half 1 (cols 256:512) — gated on the second bias-add
            sync.wait_ge(s_out, 2)
            sync.dma_start(
                out=out_d[:, 256:512], in_=out_sb[:, 256:512]
            ).then_inc(s_od, 16)
            sync.wait_ge(s_od, 32)

        @block.scalar
        def _(scalar):
            scalar.dma_start(out=hst_sb[:, 0:1024], in_=hst_d[:, 0:1024]).then_inc(
                s_hf, 16
            )
            scalar.dma_start(
                out=hst_sb[:, 1024:2048], in_=hst_d[:, 1024:2048]
            ).then_inc(s_hb, 16)
            scalar.dma_start(
                out=bias_sb[:], in_=bias_d[:].to_broadcast((_P, 1))
            ).then_inc(s_bias, 16)
            # out half 0 (cols 0:256) — gated on the first bias-add
            scalar.wait_ge(s_out, 1)
            scalar.dma_start(out=out_d[:, 0:256], in_=out_sb[:, 0:256]).then_inc(
                s_od, 16
            )
            scalar.wait_ge(s_od, 32)

        @block.tensor
        def _(tensor):
            # HAM warmup: keep the PE busy on zeros so the clock gate opens
            # (1.2 -> 2.4 GHz) while the input DMAs are in flight.
            tensor.wait_ge(s_warm, 1)
            for _i in range(_NWARM):
                nc.tensor.matmul(
                    pwarm[:], lhsT=warm_sb[:], rhs=warm_sb[:], start=True, stop=True
                )

            def s1(hc, kc):
                return nc.tensor.matmul(
                    pgt[hc][:],
                    lhsT=w_sb[:, kc * 512 + hc * _P : kc * 512 + (hc + 1) * _P],
                    rhs=hst_sb[:, kc * 512 : kc * 512 + _P],
                    start=(kc == 0),
                    stop=(kc == _KC - 1),
                )

            def s2(j):
                tensor.wait_ge(s_gt_v, j + 1)
                return nc.tensor.matmul(
                    ps[:],
                    lhsT=gt_sb[:, j * _P : (j + 1) * _P],
                    rhs=hst_sb[:, j * 512 : (j + 1) * 512],
                    start=(j == 0),
                    stop=(j == _KC - 1),
                )

            # stage 1 on the front halves (kc 0,1) — overlaps back DMAs
            tensor.wait_ge(s_wf, 16)
            tensor.wait_ge(s_hf, 16)
            for hc in range(_KC):
                for kc in (0, 1):
                    s1(hc, kc)
            # finish each group on the back halves, interleaving stage 2
            tensor.wait_ge(s_wb, 16)
            tensor.wait_ge(s_hb, 16)
            for hc in range(_KC):
                s1(hc, 2)
                s1(hc, 3).then_inc(s_gt_pe, 1)
                if hc >= 1:
                    s2(hc - 1)
            s2(_KC - 1).then_inc(s_s2, 1)

        @block.vector
        def _(vector):
            nc.vector.memset(warm_sb[:], 0.0).then_inc(s_warm, 1)
            for hc in range(_KC):
                vector.wait_ge(s_gt_pe, hc + 1)
                nc.vector.tensor_copy(
                    gt_sb[:, hc * _P : (hc + 1) * _P], pgt[hc][:]
                ).then_inc(s_gt_v, 1)
            vector.wait_ge(s_s2, 1)
            vector.wait_ge(s_bias, 16)
            for half in range(2):
                nc.vector.tensor_scalar_add(
                    out_sb[:, half * 256 : (half + 1) * 256],
                    ps[:, half * 256 : (half + 1) * 256],
                    bias_sb[:, 0:1],
                ).then_inc(s_out, 1)

    nc.compile()
    _STATE["nc"] = nc
    return nc


def _swizzle(a):
    """(512, X) row-major -> (128, 4*X): partition p gets rows p, 128+p, ..."""
    x = a.shape[1]
    return np.ascontiguousarray(
        a.reshape(_KC, _P, x).transpose(1, 0, 2).reshape(_P, _KC * x)
    )


def _device_scores(hs, w, bias):
    """Compute S[b, i, j] = (hs_b @ W @ hs_b^T)[i, j] + bias on 8 cores."""
    global LAST_RESULTS
    import ml_dtypes
    from concourse.bass_utils import run_bass_kernel_spmd

    nc = _build_nc()
    bf = ml_dtypes.bfloat16

    w_p = _swizzle(w.astype(bf))
    hst = [np.ascontiguousarray(hs[b].T.astype(bf)) for b in range(_B)]
    perms = []
    in_maps = []
    for c in range(_NCORES):
        b, rc = divmod(c, _GROUPS)
        r0 = rc * _RB
        perm = np.concatenate(
            [np.arange(r0, r0 + _RB), np.arange(0, r0), np.arange(r0 + _RB, _L)]
        )
        perms.append(perm)
        in_maps.append(
            {
                "w": w_p,
                "hst": _swizzle(np.ascontiguousarray(hst[b][:, perm])),
                "bias": bias,
            }
        )

    kwargs = dict(TRACE_KWARGS) if TRACE else {}
    res = run_bass_kernel_spmd(
        nc, in_maps, core_ids=list(range(_NCORES)), trace=TRACE, **kwargs
    )
    LAST_RESULTS = res

    s = np.empty((_B, _L, _L), np.float32)
    for c in range(_NCORES):
        b, rc = divmod(c, _GROUPS)
        inv = np.argsort(perms[c])
        s[b, rc * _RB : (rc + 1) * _RB, :] = res.results[c]["out"][:, inv]
    return s


def kernel(hidden_states, W, b, attention_mask, special_tokens_mask):
    hs = np.ascontiguousarray(np.asarray(hidden_states, dtype=np.float32))
    w = np.ascontiguousarray(np.asarray(W, dtype=np.float32)[0])
    bias = np.asarray(b, dtype=np.float32).reshape(1)
    am = np.asarray(attention_mask)
    sm = np.asarray(special_tokens_mask)

    # Pair indices from the (constant) masks — mirrors the reference.
    aa_mask = (am[0] == 1) & (sm[0] == 0)
    aa_positions = np.nonzero(aa_mask)[0]
    n_aa = aa_positions.shape[0]
    if n_aa < 2:
        return np.zeros((hs.shape[0], 0), dtype=np.float32)
    tri_i, tri_j = np.triu_indices(n_aa, k=1)
    idx_i = aa_positions[tri_i]
    idx_j = aa_positions[tri_j]

    if hs.shape != (_B, _L, _H) or w.shape != (_H, _H):
        # Defensive fallback for unexpected shapes (never hit by the spec).
        g = hs @ w
        s = np.einsum("bik,bjk->bij", g, hs) + bias[0]
        return s[:, idx_i, idx_j].astype(np.float32)

    s = _device_scores(hs, w, bias)  # bias already added on device
    return s[:, idx_i, idx_j].astype(np.float32)


# revision 9
# speedup vs baseline: 1.2669x; 1.0314x over previous
"""ContactMapHead bilinear pair-scoring kernel for 8 trn2 NeuronCores.

Math: for each batch b, logits[b, p] = h[b, i_p] @ W @ h[b, j_p] + bias,
where (i_p, j_p) enumerate position pairs (upper triangle, k=1, when the
masks keep every position — the general case is handled too).

This equals S_b = (h_b @ W) @ h_b^T + bias followed by a pair gather.
S_b is a 512x512 matrix per batch; total device work = two 512^3 matmuls
per batch (memory-bound at this size).

Sharding (8 cores): core c computes rows [r0, r0+128) of S_b for batch
b = c // 4, r0 = (c % 4) * 128.  W and h_b^T are replicated per core; the
host assembles S (2, 512, 512) from the 8 row-blocks and gathers the
pair indices (pure unshard/reindex).

v2 (this file): all inputs in bf16 — the kernel is aggregate-HBM-DMA
bound (all 8 cores share the chip's 16 SDMA engines at ~2.5 TB/s), so
halving input bytes halves the dominant phase; bf16 also makes every
matmul single-pass (fp32 runs LOW/HIGH double-pumped on the PE).
The row-slice tensor (stage-1 rhs) is folded into hst by a per-core
column permutation (cols r0..r0+127 first within each 512-col h-chunk),
so it is not transferred twice; the host inverts the permutation when
assembling S.  DMA layout keeps 2KB-per-partition runs (SDMA throughput
is per-packet limited; runs below 2KB waste packet slots).

Device program (per core), P = 128 partitions, raw bass:
    w    (128, 2048) bf16: w[p, kc*512 + c] = W[kc*128 + p, c]
    hst  (128, 2048) bf16: hst[p, hc*512 + j] = h_b[perm[j], hc*128 + p]
                           perm = [r0..r0+128) ++ [0..r0) ++ [r0+128..512)
    bias (1,) f32
    out  (128, 512) f32:  out[m, j] = S_b[r0 + m, perm[j]] + bias

  DMAs: sync ring: w front half (kc 0,1), w back half (kc 2,3)
        scalar ring: hst front (hc 0,1), hst back (hc 2,3), bias
        (each a single 2KB-run transfer; fronts land ~halfway through)
  PE:   warmup MMs on zeros (opens the HAM clock gate 1.2 -> 2.4 GHz),
        stage 1 GT[hc] += W-tile^T x hst[kc-chunk cols 0:128] with
        kc 0,1 on the front halves (overlaps the back-half DMA), then
        kc 2,3 interleaved with stage 2 ps += gt[hc] x hst[hc-chunk].
  DVE:  gt psum -> sbuf bf16 copies, then ps + bias in two column
        halves feeding two out DMAs on the two rings.
"""

import numpy as np

_B, _L, _H = 2, 512, 512
_P = 128
_KC = _H // _P          # 4 contraction chunks
_GROUPS = 4             # row-blocks per batch
_RB = _L // _GROUPS     # 128 rows per core
_NCORES = 8
_NWARM = 24             # warmup matmuls (each ~107ns pitch at full clock)

# Dev/profiling knobs (used by test.py only; harness leaves them alone).
TRACE = False
TRACE_KWARGS = {}
LAST_RESULTS = None

_STATE = {}


def _build_nc():
    """Build (once) the raw-bass module for one core's row-block."""
    if "nc" in _STATE:
        return _STATE["nc"]

    from concourse import bacc, mybir

    f32 = mybir.dt.float32
    bf16 = mybir.dt.bfloat16
    nc = bacc.Bacc("TRN2", target_bir_lowering=False, debug=False)

    w_d = nc.dram_tensor("w", [_P, 2048], bf16, kind="ExternalInput")
    hst_d = nc.dram_tensor("hst", [_P, 2048], bf16, kind="ExternalInput")
    bias_d = nc.dram_tensor("bias", [1], f32, kind="ExternalInput")
    out_d = nc.dram_tensor("out", [_RB, _L], f32, kind="ExternalOutput")

    w_sb = nc.alloc_sbuf_tensor("w_sb", [_P, 2048], bf16)
    hst_sb = nc.alloc_sbuf_tensor("hst_sb", [_P, 2048], bf16)
    bias_sb = nc.alloc_sbuf_tensor("bias_sb", [_P, 1], f32)
    gt_sb = nc.alloc_sbuf_tensor("gt_sb", [_P, 512], bf16)
    out_sb = nc.alloc_sbuf_tensor("out_sb", [_P, _L], f32)
    warm_sb = nc.alloc_sbuf_tensor("warm_sb", [_P, _P], bf16)
    pgt = [nc.alloc_psum_tensor(f"pgt{h}", [_P, _P], f32) for h in range(_KC)]
    ps0 = nc.alloc_psum_tensor("ps0", [_P, 256], f32)
    ps1 = nc.alloc_psum_tensor("ps1", [_P, 256], f32)
    pwarm = nc.alloc_psum_tensor("pwarm", [_P, _P], f32)

    s_wf = nc.alloc_semaphore("s_wf")      # +16 w front
    s_wb = nc.alloc_semaphore("s_wb")      # +16 w back
    s_hf = nc.alloc_semaphore("s_hf")      # +16 hst front
    s_hb = nc.alloc_semaphore("s_hb")      # +16 hst back
    s_bias = nc.alloc_semaphore("s_bias")  # +16 bias
    s_gt_pe = nc.alloc_semaphore("s_gt_pe")  # +1 per stage-1 group done
    s_gt_v = nc.alloc_semaphore("s_gt_v")    # +1 per gt copy
    s_s2a = nc.alloc_semaphore("s_s2a")    # +1 ps0 (cols 0:256) done
    s_s2b = nc.alloc_semaphore("s_s2b")    # +1 ps1 (cols 256:512) done
    s_out0 = nc.alloc_semaphore("s_out0")  # +1 bias-add half 0
    s_out1 = nc.alloc_semaphore("s_out1")  # +1 bias-add half 1
    s_od = nc.alloc_semaphore("s_od")      # +16 per out-DMA half
    s_warm = nc.alloc_semaphore("s_warm")  # +1 warmup scratch zeroed

    with nc.Block(no_gpsimd_drain=True) as block:

        @block.sync
        def _(sync):
            sync.dma_start(out=w_sb[:, 0:1024], in_=w_d[:, 0:1024]).then_inc(
                s_wf, 16
            )
            # out half 1 (cols 256:512) — gated on the Act-engine bias-add
            sync.wait_ge(s_out1, 1)
            sync.dma_start(
                out=out_d[:, 256:512], in_=out_sb[:, 256:512]
            ).then_inc(s_od, 16)
            sync.wait_ge(s_od, 32)

        @block.scalar
        def _(scalar):
            scalar.dma_start(out=hst_sb[:, 0:1024], in_=hst_d[:, 0:1024]).then_inc(
                s_hf, 16
            )
            # bias-add half 1 on the Act engine (runs parallel to DVE's half 0)
            scalar.wait_ge(s_s2b, 1)
            scalar.wait_ge(s_bias, 16)
            nc.scalar.activation(
                out_sb[:, 256:512],
                ps1[:],
                mybir.ActivationFunctionType.Identity,
                bias=bias_sb[:, 0:1],
            ).then_inc(s_out1, 1)
            # out half 0 (cols 0:256) — gated on the DVE bias-add
            scalar.wait_ge(s_out0, 1)
            scalar.dma_start(out=out_d[:, 0:256], in_=out_sb[:, 0:256]).then_inc(
                s_od, 16
            )
            scalar.wait_ge(s_od, 32)

        @block.tensor
        def _(tensor):
            # HAM warmup: keep the PE busy on zeros so the clock gate opens
            # (1.2 -> 2.4 GHz) while the input DMAs are in flight.
            tensor.wait_ge(s_warm, 1)
            for _i in range(_NWARM):
                nc.tensor.matmul(
                    pwarm[:], lhsT=warm_sb[:], rhs=warm_sb[:], start=True, stop=True
                )

            def s1(hc, kc):
                return nc.tensor.matmul(
                    pgt[hc][:],
                    lhsT=w_sb[:, kc * 512 + hc * _P : kc * 512 + (hc + 1) * _P],
                    rhs=hst_sb[:, kc * 512 : kc * 512 + _P],
                    start=(kc == 0),
                    stop=(kc == _KC - 1),
                )

            def s2(j):
                # two psum banks (column halves) so the bias-adds + out DMAs
                # can start before the second half finishes
                tensor.wait_ge(s_gt_v, j + 1)
                mm_a = nc.tensor.matmul(
                    ps0[:],
                    lhsT=gt_sb[:, j * _P : (j + 1) * _P],
                    rhs=hst_sb[:, j * 512 : j * 512 + 256],
                    start=(j == 0),
                    stop=(j == _KC - 1),
                )
                mm_b = nc.tensor.matmul(
                    ps1[:],
                    lhsT=gt_sb[:, j * _P : (j + 1) * _P],
                    rhs=hst_sb[:, j * 512 + 256 : (j + 1) * 512],
                    start=(j == 0),
                    stop=(j == _KC - 1),
                )
                if j == _KC - 1:
                    mm_a.then_inc(s_s2a, 1)
                    mm_b.then_inc(s_s2b, 1)

            # group 0,1 first (all kc), interleave stage-2 as gt chunks land
            tensor.wait_ge(s_wf, 16)
            tensor.wait_ge(s_hf, 16)
            tensor.wait_ge(s_wb, 16)
            tensor.wait_ge(s_hb, 16)
            for hc in range(_KC):
                for kc in range(_KC):
                    mm = s1(hc, kc)
                mm.then_inc(s_gt_pe, 1)
                if hc >= 2:
                    s2(hc - 2)
            s2(_KC - 2)
            s2(_KC - 1)

        @block.vector
        def _(vector):
            for hc in range(_KC):
                vector.wait_ge(s_gt_pe, hc + 1)
                nc.vector.tensor_copy(
                    gt_sb[:, hc * _P : (hc + 1) * _P], pgt[hc][:]
                ).then_inc(s_gt_v, 1)
            vector.wait_ge(s_s2a, 1)
            vector.wait_ge(s_bias, 16)
            nc.vector.tensor_scalar_add(
                out_sb[:, 0:256], ps0[:], bias_sb[:, 0:1]
            ).then_inc(s_out0, 1)

        @block.gpsimd
        def _(gpsimd):
            nc.gpsimd.memset(warm_sb[:], 0.0).then_inc(s_warm, 1)
            gpsimd.dma_start(out=w_sb[:, 1024:2048], in_=w_d[:, 1024:2048]).then_inc(
                s_wb, 16
            )
            gpsimd.dma_start(
                out=hst_sb[:, 1024:2048], in_=hst_d[:, 1024:2048]
            ).then_inc(s_hb, 16)
            gpsimd.dma_start(
                out=bias_sb[:], in_=bias_d[:].to_broadcast((_P, 1))
            ).then_inc(s_bias, 16)

    nc.compile()
    _STATE["nc"] = nc
    return nc


def _swizzle(a):
    """(512, X) row-major -> (128, 4*X): partition p gets rows p, 128+p, ..."""
    x = a.shape[1]
    return np.ascontiguousarray(
        a.reshape(_KC, _P, x).transpose(1, 0, 2).reshape(_P, _KC * x)
    )


def _device_scores(hs, w, bias):
    """Compute S[b, i, j] = (hs_b @ W @ hs_b^T)[i, j] + bias on 8 cores."""
    global LAST_RESULTS
    import ml_dtypes
    from concourse.bass_utils import run_bass_kernel_spmd

    nc = _build_nc()
    bf = ml_dtypes.bfloat16

    w_p = _swizzle(w.astype(bf))
    hst = [np.ascontiguousarray(hs[b].T.astype(bf)) for b in range(_B)]
    perms = []
    in_maps = []
    for c in range(_NCORES):
        b, rc = divmod(c, _GROUPS)
        r0 = rc * _RB
        perm = np.concatenate(
            [np.arange(r0, r0 + _RB), np.arange(0, r0), np.arange(r0 + _RB, _L)]
        )
        perms.append(perm)
        in_maps.append(
            {
                "w": w_p,
                "hst": _swizzle(np.ascontiguousarray(hst[b][:, perm])),
                "bias": bias,
            }
        )

    kwargs = dict(TRACE_KWARGS) if TRACE else {}
    res = run_bass_kernel_spmd(
        nc, in_maps, core_ids=list(range(_NCORES)), trace=TRACE, **kwargs
    )
    LAST_RESULTS = res

    s = np.empty((_B, _L, _L), np.float32)
    for c in range(_NCORES):
        b, rc = divmod(c, _GROUPS)
        inv = np.argsort(perms[c])
        s[b, rc * _RB : (rc + 1) * _RB, :] = res.results[c]["out"][:, inv]
    return s


def kernel(hidden_states, W, b, attention_mask, special_tokens_mask):
    hs = np.ascontiguousarray(np.asarray(hidden_states, dtype=np.float32))
    w = np.ascontiguousarray(np.asarray(W, dtype=np.float32)[0])
    bias = np.asarray(b, dtype=np.float32).reshape(1)
    am = np.asarray(attention_mask)
    sm = np.asarray(special_tokens_mask)

    # Pair indices from the (constant) masks — mirrors the reference.
    aa_mask = (am[0] == 1) & (sm[0] == 0)
    aa_positions = np.nonzero(aa_mask)[0]
    n_aa = aa_positions.shape[0]
    if n_aa < 2:
        return np.zeros((hs.shape[0], 0), dtype=np.float32)
    tri_i, tri_j = np.triu_indices(n_aa, k=1)
    idx_i = aa_positions[tri_i]
    idx_j = aa_positions[tri_j]

    if hs.shape != (_B, _L, _H) or w.shape != (_H, _H):
        # Defensive fallback for unexpected shapes (never hit by the spec).
        g = hs @ w
        s = np.einsum("bik,bjk->bij", g, hs) + bias[0]
        return s[:, idx_i, idx_j].astype(np.float32)

    s = _device_scores(hs, w, bias)  # bias already added on device
    return s[:, idx_i, idx_j].astype(np.float32)
